# revision 5
# baseline (speedup 1.0000x reference)
"""DNRI MLP decoder kernel for 8 Trainium2 NeuronCores.

Strategy (data-parallel on batch, 8 batches/core):
  - Dense 64x64 [recv, send] edge grid (4096 items, recv-major); per-edge
    weights scattered host-side into wg (duplicates/self-loops exact via
    np.add.at; absent pairs weigh 0).
  - fc1 computed as a selection-matrix matmul: ps1 = [uT; vT]^T @ S, where
    uT/vT are tiny per-batch transforms of the node features and
    S[k, (r,s)] = [k==r] + [k==64+s] is a constant 0/1 matrix. This removes
    the per-edge gather DMA entirely (no 65x4096 "pre" assembly).
  - fc1 drain: ACT relu (bias b1 rides inside uT via the ones row).
  - fc2 drain: one custom DVE op per chunk fusing relu + b2 bias + per-edge
    weight multiply + running prefix-sum; a free-dim step-0 output AP keeps
    only each 64-column group's boundary prefix, so the scatter-add over
    senders collapses into the drain. Group sums recovered by a small
    shifted subtract; type-sum done on the tiny [128, 64] aggregates.
  - Per-edge weight rows are replicated across partitions by broadcast DMA
    (idle DMA engines), consumed as the scan's Src1.
  - Output head batched across all 8 batches (single N=512 matmuls).
"""

import sys

import numpy as np

if "/opt/trn_rl_repo" not in sys.path:
    sys.path.insert(0, "/opt/trn_rl_repo")

import ml_dtypes  # noqa: E402

import concourse.bass as bass  # noqa: E402
import concourse.bacc as bacc  # noqa: E402
import concourse.mybir as mybir  # noqa: E402
from concourse import tile  # noqa: E402

NUM_VARS = 64
HID = 128
IN_F = 32
BATCH = 64
N_CORES = 8
BC = BATCH // N_CORES  # batches per core
NT = 3  # edge types used (SKIP_FIRST drops type 0)
GR = NUM_VARS * NUM_VARS  # 4096 grid items per batch
CHUNK = 2048  # columns per psum tile (4 banks)
NCHUNK = GR // CHUNK  # 2
RG = CHUNK // NUM_VARS  # recv groups per chunk (32)

F32 = mybir.dt.float32
BF16 = mybir.dt.bfloat16
NP_CDT = ml_dtypes.bfloat16

_CACHED = {}


def _register_scan_op():
    """Custom DVE op: running prefix sum of relu(in0 + s0) * in1.

    Emitted through a step-0 inner output AP, the last write per group
    keeps the group-boundary prefix => fused weighted segmented reduce."""
    import numpy as _np

    from concourse import dve_ops as _do
    from concourse.dve_spec import (
        Spec, Src0, Src1, C0, relu, scan, AluOp, lower as _lower,
    )
    from concourse.dve_uop import DveOpSpec
    from concourse.dve_ops import DveOp, has_src1

    name = "RELU_BIAS_MUL_SCAN_K77"
    if any(op.name == name for op in _do.OPS):
        return next(op for op in _do.OPS if op.name == name)

    def ref(in0, in1, s0, s1, imm2):
        s0r = _np.asarray(s0, _np.float32).reshape(
            s0.shape[0], *([1] * (in0.ndim - 1))
        )
        body = _np.maximum(in0.astype(_np.float32) + s0r, 0) * in1.astype(
            _np.float32
        )
        P = in0.shape[0]
        return _np.cumsum(body.reshape(P, -1), axis=1).reshape(body.shape)

    spec = Spec(body=scan(AluOp.ADD, relu(Src0 + C0) * Src1), reference=ref)
    op = DveOp(name, spec, subdim=False, uops_sha={})
    opcode = _do._CUSTOM_DVE_ROW_BASE + len(_do.OPS)
    _do.OPS.append(op)
    _do.CUSTOM_DVE_SPECS[name] = spec
    _do._SUB_OPCODE_FOR_NAME[name] = opcode
    for ver in ("v3", "v4"):
        try:
            s = DveOpSpec(
                name=name, opcode=opcode, uops=_lower(spec, ver=ver),
                rd1_en=has_src1(spec),
            )
            op.uops_sha[ver] = s.sha(ver)
        except Exception:
            pass
    return op


def build_kernel():
    scan_op = _register_scan_op()
    nc = bacc.Bacc("TRN2", target_bir_lowering=False)

    xTa_d = nc.dram_tensor("xTa", [IN_F + 1, BC * NUM_VARS], BF16, kind="ExternalInput")
    wg_d = nc.dram_tensor("wg", [BC, NT, GR], BF16, kind="ExternalInput")
    S_d = nc.dram_tensor("Smat", [HID, GR], BF16, kind="ExternalInput")
    W1r_d = nc.dram_tensor("W1r", [NT, IN_F + 1, HID], BF16, kind="ExternalInput")
    W1s_d = nc.dram_tensor("W1s", [NT, IN_F, HID], BF16, kind="ExternalInput")
    W2T_d = nc.dram_tensor("W2T", [NT, HID, HID], BF16, kind="ExternalInput")
    b2_d = nc.dram_tensor("b2", [NT, HID, 1], F32, kind="ExternalInput")
    O1x_d = nc.dram_tensor("O1x", [IN_F + 1, HID], BF16, kind="ExternalInput")
    O1m_d = nc.dram_tensor("O1m", [HID, HID], BF16, kind="ExternalInput")
    O2T_d = nc.dram_tensor("O2T", [HID, HID], BF16, kind="ExternalInput")
    bo2_d = nc.dram_tensor("bo2", [HID, 1], F32, kind="ExternalInput")
    muT_d = nc.dram_tensor("muT", [HID, IN_F], BF16, kind="ExternalInput")
    xres_d = nc.dram_tensor("xres", [NUM_VARS, BC * IN_F], F32, kind="ExternalInput")
    out_d = nc.dram_tensor("out", [NUM_VARS, BC * IN_F], F32, kind="ExternalOutput")

    AL = mybir.AluOpType
    RELU = mybir.ActivationFunctionType.Relu
    COPY = mybir.ActivationFunctionType.Copy

    with tile.TileContext(nc) as tc:
        with (
            tc.tile_pool(name="const", bufs=1) as cpool,
            tc.tile_pool(name="wb", bufs=2) as wbpool,
            tc.tile_pool(name="m1", bufs=2) as m1pool,
            tc.tile_pool(name="uv", bufs=2) as uvpool,
            tc.tile_pool(name="sm", bufs=2) as smpool,
            tc.tile_pool(name="ps", bufs=2, space="PSUM") as pspool,
        ):
            # ---- constants ----
            S_sb = cpool.tile([HID, GR], BF16, tag="S")
            W1r_sb = cpool.tile([IN_F + 1, NT * HID], BF16, tag="W1r")
            W1s_sb = cpool.tile([IN_F, NT * HID], BF16, tag="W1s")
            W2T_sb = cpool.tile([HID, NT * HID], BF16, tag="W2T")
            b2_sb = cpool.tile([HID, NT], F32, tag="b2")
            O1x_sb = cpool.tile([IN_F + 1, HID], BF16, tag="O1x")
            O1m_sb = cpool.tile([HID, HID], BF16, tag="O1m")
            O2T_sb = cpool.tile([HID, HID], BF16, tag="O2T")
            bo2_sb = cpool.tile([HID, 1], F32, tag="bo2")
            muT_sb = cpool.tile([HID, IN_F], BF16, tag="muT")
            xTa_sb = cpool.tile([IN_F + 1, BC * NUM_VARS], BF16, tag="xTa")
            xres_sb = cpool.tile([NUM_VARS, BC * IN_F], F32, tag="xres")
            agg_sb = cpool.tile([HID, BC * NUM_VARS], BF16, tag="agg")

            nc.sync.dma_start(S_sb[:], S_d[:])
            for i in range(NT):
                nc.sync.dma_start(W1r_sb[:, i * HID:(i + 1) * HID], W1r_d[i])
                nc.sync.dma_start(W1s_sb[:, i * HID:(i + 1) * HID], W1s_d[i])
                nc.sync.dma_start(W2T_sb[:, i * HID:(i + 1) * HID], W2T_d[i])
                nc.sync.dma_start(b2_sb[:, i:i + 1], b2_d[i])
            nc.sync.dma_start(O1x_sb[:], O1x_d[:])
            nc.sync.dma_start(O1m_sb[:], O1m_d[:])
            nc.sync.dma_start(O2T_sb[:], O2T_d[:])
            nc.sync.dma_start(bo2_sb[:], bo2_d[:])
            nc.sync.dma_start(muT_sb[:], muT_d[:])
            nc.gpsimd.dma_start(xTa_sb[:], xTa_d[:])
            nc.gpsimd.dma_start(xres_sb[:], xres_d[:])

            wdma = [nc.gpsimd, nc.scalar, nc.sync]

            for b in range(BC):
                xT_b = xTa_sb[:, b * NUM_VARS:(b + 1) * NUM_VARS]

                # per-edge weight rows broadcast across partitions (DMA bcast)
                wbs = []
                for i in range(NT):
                    wb = wbpool.tile([HID, GR], BF16, tag=f"wb{i}")
                    wdma[i].dma_start(
                        wb[:], wg_d[b, i].unsqueeze(0).to_broadcast([HID, GR])
                    )
                    wbs.append(wb)

                # ---- uT/vT: tiny feature transforms, stacked on partitions
                ps_uv = pspool.tile([HID, CHUNK], F32, tag="ps")
                for i in range(NT):
                    nc.tensor.matmul(
                        ps_uv[0:NUM_VARS, i * HID:(i + 1) * HID],
                        xT_b,
                        W1r_sb[:, i * HID:(i + 1) * HID],
                    )
                    nc.tensor.matmul(
                        ps_uv[NUM_VARS:2 * NUM_VARS, i * HID:(i + 1) * HID],
                        xT_b[0:IN_F, :],
                        W1s_sb[:, i * HID:(i + 1) * HID],
                    )
                UV = uvpool.tile([HID, NT * HID], BF16, tag="UV")
                nc.scalar.activation(UV[:], ps_uv[:, 0:NT * HID], COPY)

                # Zs[t]: per-chunk boundary prefixes (col 0 and 1+RG zeroed)
                Zs = smpool.tile([HID, NT * (2 * RG + 2)], F32, tag="Zs")
                nc.gpsimd.memset(Zs[:], 0.0)

                for c in range(NCHUNK):
                    c0 = c * CHUNK
                    m1s = []
                    for i in range(NT):
                        ps1 = pspool.tile([HID, CHUNK], F32, tag="ps")
                        for h in range(CHUNK // 512):
                            nc.tensor.matmul(
                                ps1[:, h * 512:(h + 1) * 512],
                                UV[:, i * HID:(i + 1) * HID],
                                S_sb[:, c0 + h * 512:c0 + (h + 1) * 512],
                            )
                        m1 = m1pool.tile([HID, CHUNK], BF16, tag=f"m1_{i}")
                        nc.scalar.activation(m1[:], ps1[:], RELU)
                        m1s.append(m1)
                    for i in range(NT):
                        ps2 = pspool.tile([HID, CHUNK], F32, tag="ps")
                        for h in range(CHUNK // 512):
                            nc.tensor.matmul(
                                ps2[:, h * 512:(h + 1) * 512],
                                W2T_sb[:, i * HID:(i + 1) * HID],
                                m1s[i][:, h * 512:(h + 1) * 512],
                            )
                        # fused relu + b2 + weight + prefix-sum; step-0 output
                        # keeps each 64-col group's boundary prefix
                        zcol = i * (2 * RG + 2) + c * (RG + 1) + 1
                        nc.vector._custom_dve(
                            scan_op,
                            out=Zs[:, zcol:zcol + RG]
                            .unsqueeze(2)
                            .to_broadcast([HID, RG, NUM_VARS]),
                            in0=ps2[:].rearrange("p (g s) -> p g s", g=RG),
                            in1=wbs[i][:, c0:c0 + CHUNK]
                            .rearrange("p (g s) -> p g s", g=RG),
                            s0=b2_sb[:, i:i + 1],
                        )

                # type-sum of boundary prefixes (incl. the zero cols), then
                # shifted subtract -> per-recv aggregates
                Zv = Zs[:].rearrange("p (t z) -> p t z", t=NT)
                T = smpool.tile([HID, 2 * RG + 2], F32, tag="T")
                nc.gpsimd.tensor_tensor(T[:], Zv[:, 0, :], Zv[:, 1, :], AL.add)
                nc.gpsimd.tensor_tensor(T[:], T[:], Zv[:, 2, :], AL.add)
                Tv = T[:].rearrange("p (c z) -> p c z", c=2)
                av = agg_sb[:, b * NUM_VARS:(b + 1) * NUM_VARS]
                nc.vector.tensor_tensor(
                    av.rearrange("p (c g) -> p c g", c=2),
                    Tv[:, :, 1:RG + 1],
                    Tv[:, :, 0:RG],
                    AL.subtract,
                )

            # ---- output head, batched across all BC batches ----
            pso1 = pspool.tile([HID, CHUNK], F32, tag="ps")
            nc.tensor.matmul(
                pso1[:, 0:BC * NUM_VARS], O1x_sb[:], xTa_sb[:],
                start=True, stop=False,
            )
            nc.tensor.matmul(
                pso1[:, 0:BC * NUM_VARS], O1m_sb[:], agg_sb[:],
                start=False, stop=True,
            )
            pred1 = uvpool.tile([HID, BC * NUM_VARS], BF16, tag="pred1")
            nc.scalar.activation(pred1[:], pso1[:, 0:BC * NUM_VARS], RELU)
            pso2 = pspool.tile([HID, CHUNK], F32, tag="ps")
            nc.tensor.matmul(pso2[:, 0:BC * NUM_VARS], O2T_sb[:], pred1[:])
            pred2 = uvpool.tile([HID, BC * NUM_VARS], BF16, tag="pred2")
            nc.scalar.activation(
                pred2[:], pso2[:, 0:BC * NUM_VARS], RELU, bias=bo2_sb[:]
            )
            psmu = pspool.tile([HID, CHUNK], F32, tag="ps")
            for b in range(BC):
                nc.tensor.matmul(
                    psmu[0:NUM_VARS, b * IN_F:(b + 1) * IN_F],
                    pred2[:, b * NUM_VARS:(b + 1) * NUM_VARS],
                    muT_sb[:],
                )
            out_sb = smpool.tile([NUM_VARS, BC * IN_F], F32, tag="out")
            nc.vector.tensor_tensor(
                out_sb[:], psmu[0:NUM_VARS, 0:BC * IN_F], xres_sb[:], AL.add
            )
            nc.gpsimd.dma_start(out_d[:], out_sb[:])

    nc.finalize()
    return nc


def prep_inputs(inputs, edges, msg_fc1_w, msg_fc1_b, msg_fc2_w, msg_fc2_b,
                out_fc1_w, out_fc1_b, out_fc2_w, out_fc2_b,
                mu_w, mu_b, logstd_w, logstd_b, send_edges, recv_edges):
    """Build the per-core input maps (host-side shard + repack)."""
    inputs = np.asarray(inputs, np.float32)
    edges = np.asarray(edges, np.float32)
    send = np.asarray(send_edges, np.int64)
    recv = np.asarray(recv_edges, np.int64)

    B = inputs.shape[0]
    # dense [recv, send] weight grid per (batch, type); np.add.at handles
    # duplicate (send, recv) pairs exactly
    wg = np.zeros((B, NT, GR), np.float32)
    idx = recv * NUM_VARS + send
    ed = edges[:, :, 1:1 + NT].transpose(0, 2, 1).reshape(B * NT, -1)
    np.add.at(wg.reshape(B * NT, -1), (slice(None), idx), ed)

    # constant 0/1 selection matrix: col (r,s) -> rows r and 64+s
    S = np.zeros((HID, GR), np.float32)
    cols = np.arange(GR)
    S[cols // NUM_VARS, cols] += 1.0
    S[NUM_VARS + cols % NUM_VARS, cols] += 1.0

    ones_b = np.ones((B, 1, NUM_VARS), np.float32)
    xTa = np.concatenate([inputs.transpose(0, 2, 1), ones_b], axis=1)  # [B,33,64]

    W1r = np.concatenate(
        [msg_fc1_w[1:, :, :IN_F].transpose(0, 2, 1), msg_fc1_b[1:, None, :]],
        axis=1,
    )  # [3,33,128]
    W1s = msg_fc1_w[1:, :, IN_F:].transpose(0, 2, 1)  # [3,32,128]
    W2T = msg_fc2_w[1:].transpose(0, 2, 1)  # [3,128,128]
    b2 = np.ascontiguousarray(msg_fc2_b[1:, :, None], np.float32)
    O1x = np.concatenate([out_fc1_w[:, :IN_F].T, out_fc1_b[None, :]], axis=0)
    O1m = np.ascontiguousarray(out_fc1_w[:, IN_F:].T)
    O2T = np.ascontiguousarray(out_fc2_w.T)
    bo2 = np.ascontiguousarray(out_fc2_b[:, None], np.float32)
    muT = np.ascontiguousarray(mu_w.T)

    def c(a):
        return np.ascontiguousarray(a, dtype=NP_CDT)

    shared = {
        "Smat": c(S), "W1r": c(W1r), "W1s": c(W1s), "W2T": c(W2T),
        "b2": b2, "O1x": c(O1x), "O1m": c(O1m), "O2T": c(O2T),
        "bo2": bo2, "muT": c(muT),
    }
    in_maps = []
    for core in range(N_CORES):
        lo, hi = core * BC, (core + 1) * BC
        m = dict(shared)
        # [33, BC*64]: per-batch xT slabs side by side
        m["xTa"] = c(
            xTa[lo:hi].transpose(1, 0, 2).reshape(IN_F + 1, BC * NUM_VARS)
        )
        m["wg"] = c(wg[lo:hi])
        # [64, BC*32] residual (+ mu bias folded in)
        m["xres"] = np.ascontiguousarray(
            (inputs[lo:hi] + mu_b[None, None, :])
            .transpose(1, 0, 2).reshape(NUM_VARS, BC * IN_F),
            np.float32,
        )
        in_maps.append(m)
    return in_maps


def kernel(**inputs):
    from concourse.bass_utils import run_bass_kernel_spmd

    if "nc" not in _CACHED:
        _CACHED["nc"] = build_kernel()
    nc = _CACHED["nc"]
    in_maps = prep_inputs(**inputs)
    res = run_bass_kernel_spmd(nc, in_maps, core_ids=list(range(N_CORES)))
    outs = []
    for r in res.results:
        o = r["out"].reshape(NUM_VARS, BC, IN_F).transpose(1, 0, 2)
        outs.append(o)
    return np.ascontiguousarray(np.concatenate(outs, axis=0), np.float32)


# revision 10
# speedup vs baseline: 1.4168x; 1.4168x over previous
"""DNRI MLP decoder kernel for 8 Trainium2 NeuronCores.

Strategy (data-parallel on batch, 8 batches/core):
  - Dense 64x64 [recv, send] edge grid (4096 items, recv-major); per-edge
    weights scattered host-side into wg (duplicates/self-loops exact via
    np.add.at; absent pairs weigh 0).
  - fc1 computed as a selection-matrix matmul: ps1 = [uT; vT]^T @ S, where
    uT/vT are tiny per-batch transforms of the node features and
    S[k, (r,s)] = [k==r] + [k==64+s] is a constant 0/1 matrix. This removes
    the per-edge gather DMA entirely (no 65x4096 "pre" assembly).
  - fc1 drain: ACT relu (bias b1 rides inside uT via the ones row).
  - fc2 drain: one custom DVE op per chunk fusing relu + b2 bias + per-edge
    weight multiply + running prefix-sum; a free-dim step-0 output AP keeps
    only each 64-column group's boundary prefix, so the scatter-add over
    senders collapses into the drain. Group sums recovered by a small
    shifted subtract; type-sum done on the tiny [128, 64] aggregates.
  - Per-edge weight rows are replicated across partitions by broadcast DMA
    (idle DMA engines), consumed as the scan's Src1.
  - Output head batched across all 8 batches (single N=512 matmuls).
"""

import sys

import numpy as np

if "/opt/trn_rl_repo" not in sys.path:
    sys.path.insert(0, "/opt/trn_rl_repo")

import ml_dtypes  # noqa: E402

import concourse.bass as bass  # noqa: E402
import concourse.bacc as bacc  # noqa: E402
import concourse.mybir as mybir  # noqa: E402
from concourse import tile  # noqa: E402

NUM_VARS = 64
HID = 128
IN_F = 32
BATCH = 64
N_CORES = 8
BC = BATCH // N_CORES  # batches per core
NT = 3  # edge types used (SKIP_FIRST drops type 0)
GR = NUM_VARS * NUM_VARS  # 4096 grid items per batch
CHUNK = 1024  # columns per psum tile (2 banks)
NCHUNK = GR // CHUNK  # 4
RG = CHUNK // NUM_VARS  # recv groups per chunk (16)
ZW = NCHUNK * (RG + 1)  # Zs columns per type (68)

F32 = mybir.dt.float32
BF16 = mybir.dt.bfloat16
NP_CDT = ml_dtypes.bfloat16

_CACHED = {}


def _register_scan_op():
    """Custom DVE op: running prefix sum of relu(in0 + s0) * in1.

    Emitted through a step-0 inner output AP, the last write per group
    keeps the group-boundary prefix => fused weighted segmented reduce."""
    import numpy as _np

    from concourse import dve_ops as _do
    from concourse.dve_spec import (
        Spec, Src0, Src1, C0, relu, scan, AluOp, lower as _lower,
    )
    from concourse.dve_uop import DveOpSpec
    from concourse.dve_ops import DveOp, has_src1

    name = "RELU_BIAS_MUL_SCAN_K77"
    if any(op.name == name for op in _do.OPS):
        return next(op for op in _do.OPS if op.name == name)

    def ref(in0, in1, s0, s1, imm2):
        s0r = _np.asarray(s0, _np.float32).reshape(
            s0.shape[0], *([1] * (in0.ndim - 1))
        )
        body = _np.maximum(in0.astype(_np.float32) + s0r, 0) * in1.astype(
            _np.float32
        )
        P = in0.shape[0]
        return _np.cumsum(body.reshape(P, -1), axis=1).reshape(body.shape)

    spec = Spec(body=scan(AluOp.ADD, relu(Src0 + C0) * Src1), reference=ref)
    op = DveOp(name, spec, subdim=False, uops_sha={})
    opcode = _do._CUSTOM_DVE_ROW_BASE + len(_do.OPS)
    _do.OPS.append(op)
    _do.CUSTOM_DVE_SPECS[name] = spec
    _do._SUB_OPCODE_FOR_NAME[name] = opcode
    for ver in ("v3", "v4"):
        try:
            s = DveOpSpec(
                name=name, opcode=opcode, uops=_lower(spec, ver=ver),
                rd1_en=has_src1(spec),
            )
            op.uops_sha[ver] = s.sha(ver)
        except Exception:
            pass
    return op


def build_kernel():
    scan_op = _register_scan_op()
    nc = bacc.Bacc("TRN2", target_bir_lowering=False)

    xTa_d = nc.dram_tensor("xTa", [IN_F + 1, BC * NUM_VARS], BF16, kind="ExternalInput")
    wg_d = nc.dram_tensor("wg", [BC, NT, GR], BF16, kind="ExternalInput")
    S_d = nc.dram_tensor("Smat", [HID, GR], BF16, kind="ExternalInput")
    W1r_d = nc.dram_tensor("W1r", [NT, IN_F + 1, HID], BF16, kind="ExternalInput")
    W1s_d = nc.dram_tensor("W1s", [NT, IN_F, HID], BF16, kind="ExternalInput")
    W2T_d = nc.dram_tensor("W2T", [NT, HID, HID], BF16, kind="ExternalInput")
    b2_d = nc.dram_tensor("b2", [NT, HID, 1], F32, kind="ExternalInput")
    O1x_d = nc.dram_tensor("O1x", [IN_F + 1, HID], BF16, kind="ExternalInput")
    O1m_d = nc.dram_tensor("O1m", [HID, HID], BF16, kind="ExternalInput")
    O2T_d = nc.dram_tensor("O2T", [HID, HID], BF16, kind="ExternalInput")
    bo2_d = nc.dram_tensor("bo2", [HID, 1], F32, kind="ExternalInput")
    muT_d = nc.dram_tensor("muT", [HID, IN_F], BF16, kind="ExternalInput")
    xres_d = nc.dram_tensor("xres", [NUM_VARS, BC * IN_F], F32, kind="ExternalInput")
    out_d = nc.dram_tensor("out", [NUM_VARS, BC * IN_F], F32, kind="ExternalOutput")

    AL = mybir.AluOpType
    RELU = mybir.ActivationFunctionType.Relu
    COPY = mybir.ActivationFunctionType.Copy

    with tile.TileContext(nc) as tc:
        with (
            tc.tile_pool(name="const", bufs=1) as cpool,
            tc.tile_pool(name="wb", bufs=3) as wbpool,
            tc.tile_pool(name="m1", bufs=2) as m1pool,
            tc.tile_pool(name="uv", bufs=2) as uvpool,
            tc.tile_pool(name="sm", bufs=2) as smpool,
            tc.tile_pool(name="ps1", bufs=2, space="PSUM") as ps1pool,
            tc.tile_pool(name="ps2", bufs=2, space="PSUM") as ps2pool,
        ):
            # ---- constants ----
            S_sb = cpool.tile([HID, GR], BF16, tag="S")
            W1r_sb = cpool.tile([IN_F + 1, NT * HID], BF16, tag="W1r")
            W1s_sb = cpool.tile([IN_F, NT * HID], BF16, tag="W1s")
            W2T_sb = cpool.tile([HID, NT * HID], BF16, tag="W2T")
            b2_sb = cpool.tile([HID, NT], F32, tag="b2")
            O1x_sb = cpool.tile([IN_F + 1, HID], BF16, tag="O1x")
            O1m_sb = cpool.tile([HID, HID], BF16, tag="O1m")
            O2T_sb = cpool.tile([HID, HID], BF16, tag="O2T")
            bo2_sb = cpool.tile([HID, 1], F32, tag="bo2")
            muT_sb = cpool.tile([HID, IN_F], BF16, tag="muT")
            xTa_sb = cpool.tile([IN_F + 1, BC * NUM_VARS], BF16, tag="xTa")
            xres_sb = cpool.tile([NUM_VARS, BC * IN_F], F32, tag="xres")
            agg_sb = cpool.tile([HID, BC * NUM_VARS], BF16, tag="agg")

            nc.sync.dma_start(S_sb[:], S_d[:])
            for i in range(NT):
                nc.sync.dma_start(W1r_sb[:, i * HID:(i + 1) * HID], W1r_d[i])
                nc.sync.dma_start(W1s_sb[:, i * HID:(i + 1) * HID], W1s_d[i])
                nc.sync.dma_start(W2T_sb[:, i * HID:(i + 1) * HID], W2T_d[i])
                nc.sync.dma_start(b2_sb[:, i:i + 1], b2_d[i])
            nc.sync.dma_start(O1x_sb[:], O1x_d[:])
            nc.sync.dma_start(O1m_sb[:], O1m_d[:])
            nc.sync.dma_start(O2T_sb[:], O2T_d[:])
            nc.sync.dma_start(bo2_sb[:], bo2_d[:])
            nc.sync.dma_start(muT_sb[:], muT_d[:])
            nc.gpsimd.dma_start(xTa_sb[:], xTa_d[:])
            nc.gpsimd.dma_start(xres_sb[:], xres_d[:])

            wdma = [nc.gpsimd, nc.scalar, nc.sync]

            for b in range(BC):
                xT_b = xTa_sb[:, b * NUM_VARS:(b + 1) * NUM_VARS]

                # per-edge weight rows broadcast across partitions (DMA bcast)
                wbs = []
                for i in range(NT):
                    wb = wbpool.tile([HID, GR], BF16, tag=f"wb{i}")
                    wdma[i].dma_start(
                        wb[:], wg_d[b, i].unsqueeze(0).to_broadcast([HID, GR])
                    )
                    wbs.append(wb)

                # ---- uT/vT: tiny feature transforms, stacked on partitions
                ps_uv = ps1pool.tile([HID, CHUNK], F32, tag="ps")
                for i in range(NT):
                    nc.tensor.matmul(
                        ps_uv[0:NUM_VARS, i * HID:(i + 1) * HID],
                        xT_b,
                        W1r_sb[:, i * HID:(i + 1) * HID],
                    )
                    nc.tensor.matmul(
                        ps_uv[NUM_VARS:2 * NUM_VARS, i * HID:(i + 1) * HID],
                        xT_b[0:IN_F, :],
                        W1s_sb[:, i * HID:(i + 1) * HID],
                    )
                UV = uvpool.tile([HID, NT * HID], BF16, tag="UV")
                nc.scalar.activation(UV[:], ps_uv[:, 0:NT * HID], COPY)

                # Zs[t]: per-chunk boundary prefixes (col c*(RG+1) zeroed)
                Zs = smpool.tile([HID, NT * ZW], F32, tag="Zs")
                nc.gpsimd.memset(Zs[:], 0.0)

                for c in range(NCHUNK):
                    c0 = c * CHUNK
                    for i in range(NT):
                        ps1 = ps1pool.tile([HID, CHUNK], F32, tag="ps")
                        for h in range(CHUNK // 512):
                            nc.tensor.matmul(
                                ps1[:, h * 512:(h + 1) * 512],
                                UV[:, i * HID:(i + 1) * HID],
                                S_sb[:, c0 + h * 512:c0 + (h + 1) * 512],
                            )
                        m1 = m1pool.tile([HID, CHUNK], BF16, tag=f"m1_{i}")
                        nc.scalar.activation(m1[:], ps1[:], RELU)
                        ps2 = ps2pool.tile([HID, CHUNK], F32, tag="ps")
                        for h in range(CHUNK // 512):
                            nc.tensor.matmul(
                                ps2[:, h * 512:(h + 1) * 512],
                                W2T_sb[:, i * HID:(i + 1) * HID],
                                m1[:, h * 512:(h + 1) * 512],
                            )
                        # fused relu + b2 + weight + prefix-sum; step-0 output
                        # keeps each 64-col group's boundary prefix
                        zcol = i * ZW + c * (RG + 1) + 1
                        nc.vector._custom_dve(
                            scan_op,
                            out=Zs[:, zcol:zcol + RG]
                            .unsqueeze(2)
                            .to_broadcast([HID, RG, NUM_VARS]),
                            in0=ps2[:].rearrange("p (g s) -> p g s", g=RG),
                            in1=wbs[i][:, c0:c0 + CHUNK]
                            .rearrange("p (g s) -> p g s", g=RG),
                            s0=b2_sb[:, i:i + 1],
                        )

                # type-sum of boundary prefixes (incl. the zero cols), then
                # shifted subtract -> per-recv aggregates
                Zv = Zs[:].rearrange("p (t z) -> p t z", t=NT)
                T = smpool.tile([HID, ZW], F32, tag="T")
                nc.gpsimd.tensor_tensor(T[:], Zv[:, 0, :], Zv[:, 1, :], AL.add)
                nc.gpsimd.tensor_tensor(T[:], T[:], Zv[:, 2, :], AL.add)
                Tv = T[:].rearrange("p (c z) -> p c z", c=NCHUNK)
                av = agg_sb[:, b * NUM_VARS:(b + 1) * NUM_VARS]
                nc.vector.tensor_tensor(
                    av.rearrange("p (c g) -> p c g", c=NCHUNK),
                    Tv[:, :, 1:RG + 1],
                    Tv[:, :, 0:RG],
                    AL.subtract,
                )

            # ---- output head, batched across all BC batches ----
            pso1 = ps1pool.tile([HID, CHUNK], F32, tag="ps")
            nc.tensor.matmul(
                pso1[:, 0:BC * NUM_VARS], O1x_sb[:], xTa_sb[:],
                start=True, stop=False,
            )
            nc.tensor.matmul(
                pso1[:, 0:BC * NUM_VARS], O1m_sb[:], agg_sb[:],
                start=False, stop=True,
            )
            pred1 = uvpool.tile([HID, BC * NUM_VARS], BF16, tag="pred1")
            nc.scalar.activation(pred1[:], pso1[:, 0:BC * NUM_VARS], RELU)
            pso2 = ps2pool.tile([HID, CHUNK], F32, tag="ps")
            nc.tensor.matmul(pso2[:, 0:BC * NUM_VARS], O2T_sb[:], pred1[:])
            pred2 = uvpool.tile([HID, BC * NUM_VARS], BF16, tag="pred2")
            nc.scalar.activation(
                pred2[:], pso2[:, 0:BC * NUM_VARS], RELU, bias=bo2_sb[:]
            )
            psmu = ps1pool.tile([HID, CHUNK], F32, tag="ps")
            for b in range(BC):
                nc.tensor.matmul(
                    psmu[0:NUM_VARS, b * IN_F:(b + 1) * IN_F],
                    pred2[:, b * NUM_VARS:(b + 1) * NUM_VARS],
                    muT_sb[:],
                )
            out_sb = smpool.tile([NUM_VARS, BC * IN_F], F32, tag="out")
            nc.vector.tensor_tensor(
                out_sb[:], psmu[0:NUM_VARS, 0:BC * IN_F], xres_sb[:], AL.add
            )
            nc.gpsimd.dma_start(out_d[:], out_sb[:])

    nc.finalize()
    return nc


def prep_inputs(inputs, edges, msg_fc1_w, msg_fc1_b, msg_fc2_w, msg_fc2_b,
                out_fc1_w, out_fc1_b, out_fc2_w, out_fc2_b,
                mu_w, mu_b, logstd_w, logstd_b, send_edges, recv_edges):
    """Build the per-core input maps (host-side shard + repack)."""
    inputs = np.asarray(inputs, np.float32)
    edges = np.asarray(edges, np.float32)
    send = np.asarray(send_edges, np.int64)
    recv = np.asarray(recv_edges, np.int64)

    B = inputs.shape[0]
    # dense [recv, send] weight grid per (batch, type); np.add.at handles
    # duplicate (send, recv) pairs exactly
    wg = np.zeros((B, NT, GR), np.float32)
    idx = recv * NUM_VARS + send
    ed = edges[:, :, 1:1 + NT].transpose(0, 2, 1).reshape(B * NT, -1)
    np.add.at(wg.reshape(B * NT, -1), (slice(None), idx), ed)

    # constant 0/1 selection matrix: col (r,s) -> rows r and 64+s
    S = np.zeros((HID, GR), np.float32)
    cols = np.arange(GR)
    S[cols // NUM_VARS, cols] += 1.0
    S[NUM_VARS + cols % NUM_VARS, cols] += 1.0

    ones_b = np.ones((B, 1, NUM_VARS), np.float32)
    xTa = np.concatenate([inputs.transpose(0, 2, 1), ones_b], axis=1)  # [B,33,64]

    W1r = np.concatenate(
        [msg_fc1_w[1:, :, :IN_F].transpose(0, 2, 1), msg_fc1_b[1:, None, :]],
        axis=1,
    )  # [3,33,128]
    W1s = msg_fc1_w[1:, :, IN_F:].transpose(0, 2, 1)  # [3,32,128]
    W2T = msg_fc2_w[1:].transpose(0, 2, 1)  # [3,128,128]
    b2 = np.ascontiguousarray(msg_fc2_b[1:, :, None], np.float32)
    O1x = np.concatenate([out_fc1_w[:, :IN_F].T, out_fc1_b[None, :]], axis=0)
    O1m = np.ascontiguousarray(out_fc1_w[:, IN_F:].T)
    O2T = np.ascontiguousarray(out_fc2_w.T)
    bo2 = np.ascontiguousarray(out_fc2_b[:, None], np.float32)
    muT = np.ascontiguousarray(mu_w.T)

    def c(a):
        return np.ascontiguousarray(a, dtype=NP_CDT)

    shared = {
        "Smat": c(S), "W1r": c(W1r), "W1s": c(W1s), "W2T": c(W2T),
        "b2": b2, "O1x": c(O1x), "O1m": c(O1m), "O2T": c(O2T),
        "bo2": bo2, "muT": c(muT),
    }
    in_maps = []
    for core in range(N_CORES):
        lo, hi = core * BC, (core + 1) * BC
        m = dict(shared)
        # [33, BC*64]: per-batch xT slabs side by side
        m["xTa"] = c(
            xTa[lo:hi].transpose(1, 0, 2).reshape(IN_F + 1, BC * NUM_VARS)
        )
        m["wg"] = c(wg[lo:hi])
        # [64, BC*32] residual (+ mu bias folded in)
        m["xres"] = np.ascontiguousarray(
            (inputs[lo:hi] + mu_b[None, None, :])
            .transpose(1, 0, 2).reshape(NUM_VARS, BC * IN_F),
            np.float32,
        )
        in_maps.append(m)
    return in_maps


def kernel(**inputs):
    from concourse.bass_utils import run_bass_kernel_spmd

    if "nc" not in _CACHED:
        _CACHED["nc"] = build_kernel()
    nc = _CACHED["nc"]
    in_maps = prep_inputs(**inputs)
    res = run_bass_kernel_spmd(nc, in_maps, core_ids=list(range(N_CORES)))
    outs = []
    for r in res.results:
        o = r["out"].reshape(NUM_VARS, BC, IN_F).transpose(1, 0, 2)
        outs.append(o)
    return np.ascontiguousarray(np.concatenate(outs, axis=0), np.float32)
